# revision 30
# baseline (speedup 1.0000x reference)
"""Multi-head attention forward on 8 Trainium2 NeuronCores.

Reference computation (B=2, N=2048, C=1024, H=16, Dh=64):
    qkv = x @ qkv_w.T + qkv_b            -> q, k, v per head
    attn = softmax(q @ k.T / sqrt(Dh))
    out  = (attn @ v) reassembled, then out @ proj_w.T + proj_b

Sharding: 8 cores = 2 batches x 4 head groups (tensor parallel on heads,
data parallel on batch).  Each core computes q/k/v for its 4 heads over
its batch's 2048 tokens, attention for those heads, and a partial
projection with its head-group's rows of proj_w.  The host sums the 4
partial projections per batch and adds the (host-folded) proj + v biases.

Schedule: everything runs in the S^T orientation (S^T[j,i] = sum_d
kT[d,j] qT[d,i]) so softmax reductions over keys happen via matmul -- a
ones column in v-hat yields the denominator as row 64 of the AV PSUM
tile.  The kernel is ACT(exp)-bound, so the S+exp stream is decoupled
from the AV stream: exp'd score tiles (es) buffer in SBUF, letting exp
run ahead across query chunks while q/k/v production and the projection
back-fill PE slack.  All matmul operands are bf16.
Softmax max-subtraction is skipped (S ~ N(0,1)).  The k bias is
dropped (softmax-invariant); v/proj biases are folded on the host.
"""

import sys

if "/opt/trn_rl_repo" not in sys.path:
    sys.path.insert(0, "/opt/trn_rl_repo")

from contextlib import ExitStack

import ml_dtypes
import numpy as np

from concourse import bacc, mybir, tile
from concourse.bass_utils import run_bass_kernel_spmd

F32 = mybir.dt.float32
F32R = mybir.dt.float32r
BF16 = mybir.dt.bfloat16
AF = mybir.ActivationFunctionType

B, N, C, H, DH = 2, 2048, 1024, 16, 64
NCORES = 8
HG = 4              # head groups (cores per batch)
HPG = H // HG       # 4 heads per core
DG = HPG * DH       # 256 projected dims per core
CT = C // 128       # 8 contraction tiles
JT = N // 128       # 16 key tiles
IC = N // 512       # 4 query chunks
SCALE = DH ** -0.5

_CACHE = {}
LAST_RESULTS = None


def _build():
    nc = bacc.Bacc("TRN2", target_bir_lowering=False, debug=False,
                   num_devices=NCORES)

    xT = nc.dram_tensor("xT", [CT, IC, 128, 512], BF16, kind="ExternalInput").ap()
    wq0 = nc.dram_tensor("wq0", [128, CT, 128], BF16, kind="ExternalInput").ap()
    wq1 = nc.dram_tensor("wq1", [128, CT, 128], BF16, kind="ExternalInput").ap()
    wk0 = nc.dram_tensor("wk0", [128, CT, 128], BF16, kind="ExternalInput").ap()
    wk1 = nc.dram_tensor("wk1", [128, CT, 128], BF16, kind="ExternalInput").ap()
    wv = nc.dram_tensor("wv", [128, CT, DG], BF16, kind="ExternalInput").ap()
    wp = nc.dram_tensor("wp", [128, DG // 128, C], BF16, kind="ExternalInput").ap()
    qbT = nc.dram_tensor("qbT", [128, 2], F32, kind="ExternalInput").ap()
    ones = nc.dram_tensor("ones", [1, 512], BF16, kind="ExternalInput").ap()
    sel = nc.dram_tensor("sel", [2, 128], F32R, kind="ExternalInput").ap()
    y = nc.dram_tensor("y", [N, C], F32, kind="ExternalOutput").ap()

    with tile.TileContext(nc) as tc, ExitStack() as ctx:
        per = ctx.enter_context(tc.tile_pool(name="per", bufs=1))
        xT_s = per.tile([128, CT, N], BF16, tag="xT")
        qT_s = per.tile([128, 2, N], BF16, tag="qT")
        kT_s = per.tile([128, 2, N], BF16, tag="kT")
        vh_s = per.tile([128, JT, HPG, DH + 1], BF16, tag="vh")
        aoT_s = per.tile([128, 2, N], BF16, tag="aoT")
        wq0_t = per.tile([128, CT, 128], BF16, tag="wq0")
        wq1_t = per.tile([128, CT, 128], BF16, tag="wq1")
        wk0_t = per.tile([128, CT, 128], BF16, tag="wk0")
        wk1_t = per.tile([128, CT, 128], BF16, tag="wk1")
        wv_t = per.tile([128, CT, DG], BF16, tag="wv")
        wp_t = per.tile([128, DG // 128, C], BF16, tag="wp")
        qbT_s = per.tile([128, 2], F32, tag="qbT")
        ones_s = per.tile([1, 512], BF16, tag="ones")
        sel_s = per.tile([2, 128], F32R, tag="sel")
        warm = per.tile([1, 16], F32, tag="warm")

        # ---- DMA emission (order = per-queue priority; x is host-tiled
        # so every chunk is one fully-contiguous descriptor) ---------------
        QS = {"sync": nc.sync, "gp": nc.gpsimd, "sc": nc.scalar}

        def xdma(q, ct, nck):
            QS[q].dma_start(xT_s[:, ct, nck * 512:(nck + 1) * 512],
                            xT[ct, nck])

        nc.sync.dma_start(ones_s[:], ones)
        nc.sync.dma_start(wk0_t[:], wk0)
        nc.gpsimd.dma_start(wq0_t[:], wq0)
        nc.scalar.dma_start(qbT_s[:], qbT)
        nc.scalar.dma_start(sel_s[:], sel)
        # first key/query chunk: ct-ordered round-robin over the queues so
        # the production matmuls (which consume in ct order) start early
        for ct, q in enumerate(["sync", "sync", "sync", "gp", "gp", "gp",
                                "sc", "sc"]):
            xdma(q, ct, 0)
        for ct in range(4):
            xdma("sync", ct, 1)
        for ct in range(4, 8):
            xdma("gp", ct, 1)
        nc.scalar.dma_start(wv_t[:], wv)
        for nck in (2, 3):
            for ct in range(4):
                xdma("sync", ct, nck)
            for ct in range(4, 8):
                xdma("gp", ct, nck)
        nc.scalar.dma_start(wk1_t[:], wk1)
        nc.scalar.dma_start(wq1_t[:], wq1)
        nc.scalar.dma_start(wp_t[:], wp)

        with tc.tile_pool(name="es", bufs=20) as esp, \
             tc.tile_pool(name="sm", bufs=2) as sm2, \
             tc.tile_pool(name="yp", bufs=2) as yp, \
             tc.tile_pool(name="psA", bufs=2, space="PSUM") as psA, \
             tc.tile_pool(name="psS", bufs=2, space="PSUM") as psS, \
             tc.tile_pool(name="psB", bufs=2, space="PSUM") as psB:

            # warm the exp table while the bulk DMAs run, and spin the
            # PE clock up to full rate (dense dummy matmuls overlap the
            # first x-chunk DMAs, so production starts warm)
            nc.scalar.activation(warm[:], ones_s[:, 0:16], AF.Exp)
            jk0 = psA.tile([128, 512], F32, tag="mm", name="jk0")
            for _ in range(8):
                nc.tensor.matmul(jk0[:], ones_s[:, 0:128], ones_s[:],
                                 start=True, stop=True)
            nc.gpsimd.memset(vh_s[:, :, :, DH], 1.0)

            def qk_chunk(w_t, dst, dt, nck, bias=False):
                ps = psA.tile([128, 512], F32, tag="mm")
                for ct in range(CT):
                    nc.tensor.matmul(
                        ps[:], w_t[:, ct, :],
                        xT_s[:, ct, nck * 512:(nck + 1) * 512],
                        start=(ct == 0), stop=(ct == CT - 1))
                out = dst[:, dt, nck * 512:(nck + 1) * 512]
                if bias:
                    nc.vector.tensor_scalar_add(out, ps[:],
                                                qbT_s[:, dt:dt + 1])
                else:
                    nc.vector.tensor_copy(out, ps[:])

            def vhat(jt):
                ps = psA.tile([128, 512], F32, tag="mm")
                for ct in range(CT):
                    nc.tensor.matmul(ps[:, 0:DG],
                                     xT_s[:, ct, jt * 128:(jt + 1) * 128],
                                     wv_t[:, ct, :],
                                     start=(ct == 0), stop=(ct == CT - 1))
                for h in range(HPG):
                    nc.vector.tensor_copy(vh_s[:, jt, h, 0:DH],
                                          ps[:, h * DH:(h + 1) * DH])

            es_store = {}
            av_tiles = {}

            def se(p, ic, jcs):
                # S^T + exp for key tiles jcs of query chunk ic
                i0 = ic * 512
                for jc in jcs:
                    st = psS.tile([128, 1024], F32, tag="st")
                    nc.tensor.matmul(st[:, 0:512],
                                     kT_s[0:64, p, jc * 128:(jc + 1) * 128],
                                     qT_s[0:64, p, i0:i0 + 512],
                                     start=True, stop=True)
                    nc.tensor.matmul(st[:, 512:1024],
                                     kT_s[64:128, p, jc * 128:(jc + 1) * 128],
                                     qT_s[64:128, p, i0:i0 + 512],
                                     start=True, stop=True)
                    es = esp.tile([128, 1024], BF16, tag="es",
                                  name=f"es{p}_{ic}_{jc}")
                    es_store[(p, ic, jc)] = es
                    nc.scalar.activation(es[:], st[:], AF.Exp, scale=SCALE)

            def av(p, ic, blk):
                if (p, ic) not in av_tiles:
                    av_tiles[(p, ic)] = [
                        psB.tile([DH + 1, 512], F32, tag="outT",
                                 name=f"o{p}_{ic}{s}") for s in "ab"]
                outs = av_tiles[(p, ic)]
                for jc in range(4 * blk, 4 * blk + 4):
                    es = es_store.pop((p, ic, jc))
                    for h in range(2):
                        nc.tensor.matmul(
                            outs[h][:], vh_s[:, jc, 2 * p + h, :],
                            es[:, h * 512:(h + 1) * 512],
                            start=(jc == 0), stop=(jc == JT - 1))

            def norm(p, ic, act_assist=False):
                # PSUM-releasing copies first (frees psB for the next
                # chunk's AVs quickly), then recip/broadcast/multiply
                i0 = ic * 512
                outs = av_tiles.pop((p, ic))
                aos, dens = [], []
                for hi, outT in enumerate(outs):
                    ao = aoT_s[hi * 64:hi * 64 + 64, p, i0:i0 + 512]
                    den = sm2.tile([1, 512], F32, tag="den")
                    nc.vector.tensor_copy(ao, outT[0:64, :])
                    nc.vector.tensor_copy(den[:], outT[64:65, :])
                    aos.append(ao)
                    dens.append(den)
                recs = []
                for hi in range(2):
                    rec = sm2.tile([1, 512], F32, tag="rec")
                    nc.vector.reciprocal_approx_fast(rec[:], dens[hi][:])
                    rec_r = sm2.tile([1, 512], F32R, tag="rec_r")
                    nc.vector.tensor_copy(rec_r[:], rec[:])
                    recs.append(rec_r)
                bcs = []
                for hi in range(2):
                    bc = psA.tile([128, 512], F32, tag="mm")
                    nc.tensor.matmul(bc[0:64, :], sel_s[0:1, 0:64], recs[hi][:],
                                     start=True, stop=True)
                    bcs.append(bc)
                for hi in range(2):
                    nc.vector.tensor_mul(aos[hi], aos[hi], bcs[hi][0:64, :])

            def proj_it(it, act_assist=False):
                # one 128-row tile of y: y[it] = aoT[:, :, it].T @ wp
                pss = [psA.tile([128, 512], F32, tag="mm", name=f"pj{it}_{e}")
                       for e in range(2)]
                for dt in range(DG // 128):
                    for ec in range(2):
                        nc.tensor.matmul(
                            pss[ec][:],
                            aoT_s[:, dt, it * 128:(it + 1) * 128],
                            wp_t[:, dt, ec * 512:(ec + 1) * 512],
                            start=(dt == 0), stop=(dt == DG // 128 - 1))
                yt = yp.tile([128, C], F32, tag="y")
                nc.vector.tensor_copy(yt[:, 0:512], pss[0][:])
                if act_assist:
                    nc.scalar.copy(yt[:, 512:1024], pss[1][:])
                else:
                    nc.vector.tensor_copy(yt[:, 512:1024], pss[1][:])
                eng = nc.sync if it % 2 == 0 else nc.gpsimd
                eng.dma_start(y[it * 128:(it + 1) * 128, :], yt[:])

            BLKS = [list(range(4 * b, 4 * b + 4)) for b in range(4)]

            # ---- emission schedule ---------------------------------------
            # buildup: S+exp streams run ahead (no vhat needed); AV trails.
            qk_chunk(wk0_t, kT_s, 0, 0)
            qk_chunk(wq0_t, qT_s, 0, 0, bias=True)
            se(0, 0, BLKS[0])
            qk_chunk(wk0_t, kT_s, 0, 1)
            se(0, 0, BLKS[1])
            qk_chunk(wq0_t, qT_s, 0, 1, bias=True)
            se(0, 1, BLKS[0])
            qk_chunk(wk0_t, kT_s, 0, 2)
            se(0, 0, BLKS[2])
            se(0, 1, BLKS[1])
            qk_chunk(wk0_t, kT_s, 0, 3)
            vhat(0), vhat(1), vhat(2), vhat(3)
            se(0, 0, BLKS[3])
            av(0, 0, 0)
            vhat(4), vhat(5), vhat(6), vhat(7)
            qk_chunk(wq0_t, qT_s, 0, 2, bias=True)
            se(0, 1, BLKS[2])
            av(0, 0, 1)
            vhat(8), vhat(9), vhat(10), vhat(11)
            qk_chunk(wq0_t, qT_s, 0, 3, bias=True)
            se(0, 1, BLKS[3])
            av(0, 0, 2)
            vhat(12), vhat(13), vhat(14), vhat(15)
            se(0, 2, BLKS[0])
            av(0, 0, 3)
            norm(0, 0)

            # steady state: se leads, av trails ~2 blocks (catching up from
            # the buildup deficit), pair-1 q/k production and the projection
            # fill the remaining PE slack.  Production must be emitted
            # before its consuming se (same in-order PE queue).
            fill = [
                lambda: qk_chunk(wk1_t, kT_s, 1, 0),
                lambda: qk_chunk(wk1_t, kT_s, 1, 1),
                lambda: qk_chunk(wq1_t, qT_s, 1, 0, bias=True),
                lambda: qk_chunk(wk1_t, kT_s, 1, 2),
                lambda: qk_chunk(wk1_t, kT_s, 1, 3),
                lambda: qk_chunk(wq1_t, qT_s, 1, 1, bias=True),
                lambda: qk_chunk(wq1_t, qT_s, 1, 2, bias=True),
                lambda: qk_chunk(wq1_t, qT_s, 1, 3, bias=True),
            ]
            seq = [(0, 2, 1), (0, 2, 2), (0, 2, 3),
                   (0, 3, 0), (0, 3, 1), (0, 3, 2), (0, 3, 3),
                   (1, 0, 0), (1, 0, 1), (1, 0, 2), (1, 0, 3),
                   (1, 1, 0), (1, 1, 1), (1, 1, 2), (1, 1, 3),
                   (1, 2, 0), (1, 2, 1), (1, 2, 2), (1, 2, 3),
                   (1, 3, 0), (1, 3, 1), (1, 3, 2), (1, 3, 3)]
            avq = [(p, ic, b)
                   for (p, ic) in [(0, 1), (0, 2), (0, 3),
                                   (1, 0), (1, 1), (1, 2), (1, 3)]
                   for b in range(4)]
            proj_pending = []
            se_done = {(0, 0, b) for b in range(4)}
            se_done |= {(0, 1, b) for b in range(4)}
            se_done.add((0, 2, 0))

            def pop_av():
                final = len(avq) == 1
                ap_, ai_, ab_ = avq.pop(0)
                av(ap_, ai_, ab_)
                if ab_ == 3:
                    if final:
                        # bridge the last norm's DVE latency with dummy
                        # matmuls so the PE clock stays at full rate for
                        # the final projection tiles
                        jk = psA.tile([128, 512], F32, tag="mm", name="jk")
                        for _ in range(14):
                            nc.tensor.matmul(jk[:], ones_s[:, 0:128],
                                             ones_s[:], start=True, stop=True)
                    norm(ap_, ai_, act_assist=final)
                    if ap_ == 1:
                        proj_pending.extend(range(4 * ai_, 4 * ai_ + 4))

            last_norm = avq[-1][:2]
            fi = 0
            for si, (pi, ici, blk) in enumerate(seq):
                if fi < len(fill) and si % 2 == 1:
                    fill[fi]()        # before se: consumers must trail
                    fi += 1
                se(pi, ici, BLKS[blk])
                se_done.add((pi, ici, blk))
                pop_av()
                # catch up: early (buildup deficit) and twice late (shrink
                # the drain tail), bounded by what se has emitted
                if (si < 3 or si in (16, 19)) and avq \
                        and tuple(avq[0]) in se_done:
                    pop_av()
                if proj_pending:
                    proj_it(proj_pending.pop(0))
            while avq:
                pop_av()
            while proj_pending:
                proj_it(proj_pending.pop(0), act_assist=True)

    nc.compile()
    return nc


def _get_nc():
    if "nc" not in _CACHE:
        _CACHE["nc"] = _build()
    return _CACHE["nc"]


def kernel(x, qkv_w, qkv_b, proj_w, proj_b):
    global LAST_RESULTS
    x = np.asarray(x, dtype=np.float32)
    qkv_w = np.asarray(qkv_w, dtype=np.float32)
    qkv_b = np.asarray(qkv_b, dtype=np.float32)
    proj_w = np.asarray(proj_w, dtype=np.float32)
    proj_b = np.asarray(proj_b, dtype=np.float32)

    nc = _get_nc()
    bf16 = ml_dtypes.bfloat16

    wqT_f = qkv_w[0:C].T                # [C, C]
    wkT_f = qkv_w[C:2 * C].T
    wvT_f = qkv_w[2 * C:3 * C].T
    wpT_f = proj_w.T                    # [C, C]

    def tile128(a):
        # [C, W] -> [128, CT, W] with partition = c % 128, ct = c // 128
        w = a.shape[1]
        return np.ascontiguousarray(
            a.reshape(CT, 128, w).transpose(1, 0, 2))

    in_maps = []
    for c in range(NCORES):
        b, g = divmod(c, HG)
        ds = g * DG
        wq_g = tile128(wqT_f[:, ds:ds + DG]).astype(bf16)  # [128, CT, 256]
        wk_g = tile128(wkT_f[:, ds:ds + DG]).astype(bf16)
        wp_g = np.ascontiguousarray(
            wpT_f[ds:ds + DG].reshape(2, 128, C).transpose(1, 0, 2)).astype(bf16)
        # qbT: per-partition q bias, column dt = head pair
        qbT = np.ascontiguousarray(
            qkv_b[ds:ds + DG].reshape(2, 128).T, dtype=np.float32)
        sel_a = np.zeros((2, 128), np.float32)
        sel_a[0, 0:64] = 1.0
        sel_a[1, 64:128] = 1.0
        in_maps.append({
            "xT": np.ascontiguousarray(
                x[b].T.reshape(CT, 128, IC, 512).transpose(0, 2, 1, 3)
            ).astype(bf16),
            "wq0": np.ascontiguousarray(wq_g[:, :, 0:128]),
            "wq1": np.ascontiguousarray(wq_g[:, :, 128:256]),
            "wk0": np.ascontiguousarray(wk_g[:, :, 0:128]),
            "wk1": np.ascontiguousarray(wk_g[:, :, 128:256]),
            "wv": tile128(wvT_f[:, ds:ds + DG]).astype(bf16),
            "wp": wp_g,
            "qbT": qbT,
            "ones": np.ones((1, 512), bf16),
            "sel": sel_a,
        })

    LAST_RESULTS = run_bass_kernel_spmd(nc, in_maps, list(range(NCORES)))
    # host unshard: sum the 4 partial projections per batch and add the
    # folded bias (proj_b + v_bias @ proj_w.T -- exact, since sum(attn)=1)
    out_bias = proj_b + qkv_b[2 * C:3 * C] @ proj_w.T
    out = np.empty((B, N, C), np.float32)
    for b in range(B):
        acc = LAST_RESULTS.results[b * HG]["y"].astype(np.float32)
        for g in range(1, HG):
            acc = acc + LAST_RESULTS.results[b * HG + g]["y"]
        out[b] = acc + out_bias
    return out


# revision 31
# speedup vs baseline: 1.1559x; 1.1559x over previous
"""Multi-head attention forward on 8 Trainium2 NeuronCores.

Reference computation (B=2, N=2048, C=1024, H=16, Dh=64):
    qkv = x @ qkv_w.T + qkv_b            -> q, k, v per head
    attn = softmax(q @ k.T / sqrt(Dh))
    out  = (attn @ v) reassembled, then out @ proj_w.T + proj_b

Sharding: 8 cores = 2 batches x 4 head groups (tensor parallel on heads,
data parallel on batch).  Each core computes q/k/v for its 4 heads over
its batch's 2048 tokens, attention for those heads, and a partial
projection with its head-group's rows of proj_w.  The host sums the 4
partial projections per batch and adds the (host-folded) proj + v biases.

Schedule: everything runs in the S^T orientation (S^T[j,i] = sum_d
kT[d,j] qT[d,i]) so softmax reductions over keys happen via matmul -- a
ones column in v-hat yields the denominator as row 64 of the AV PSUM
tile.  The kernel is ACT(exp)-bound, so the S+exp stream is decoupled
from the AV stream: exp'd score tiles (es) buffer in SBUF, letting exp
run ahead across query chunks while q/k/v production and the projection
back-fill PE slack.  All matmul operands are bf16.
Softmax max-subtraction is skipped (S ~ N(0,1)).  The k bias is
dropped (softmax-invariant); v/proj biases are folded on the host.
"""

import sys

if "/opt/trn_rl_repo" not in sys.path:
    sys.path.insert(0, "/opt/trn_rl_repo")

from contextlib import ExitStack

import ml_dtypes
import numpy as np

from concourse import bacc, mybir, tile
from concourse.bass_utils import run_bass_kernel_spmd

F32 = mybir.dt.float32
F32R = mybir.dt.float32r
BF16 = mybir.dt.bfloat16
AF = mybir.ActivationFunctionType

B, N, C, H, DH = 2, 2048, 1024, 16, 64
NCORES = 8
HG = 4              # head groups (cores per batch)
HPG = H // HG       # 4 heads per core
DG = HPG * DH       # 256 projected dims per core
CT = C // 128       # 8 contraction tiles
JT = N // 128       # 16 key tiles
IC = N // 512       # 4 query chunks
SCALE = DH ** -0.5

_CACHE = {}
LAST_RESULTS = None


def _build():
    nc = bacc.Bacc("TRN2", target_bir_lowering=False, debug=False,
                   num_devices=NCORES)

    xT = nc.dram_tensor("xT", [CT, IC, 128, 512], BF16, kind="ExternalInput").ap()
    wq0 = nc.dram_tensor("wq0", [128, CT, 128], BF16, kind="ExternalInput").ap()
    wq1 = nc.dram_tensor("wq1", [128, CT, 128], BF16, kind="ExternalInput").ap()
    wk0 = nc.dram_tensor("wk0", [128, CT, 128], BF16, kind="ExternalInput").ap()
    wk1 = nc.dram_tensor("wk1", [128, CT, 128], BF16, kind="ExternalInput").ap()
    wv = nc.dram_tensor("wv", [128, CT, DG], BF16, kind="ExternalInput").ap()
    wp = nc.dram_tensor("wp", [128, DG // 128, C], BF16, kind="ExternalInput").ap()
    qbT = nc.dram_tensor("qbT", [128, 2], F32, kind="ExternalInput").ap()
    ones = nc.dram_tensor("ones", [1, 512], BF16, kind="ExternalInput").ap()
    sel = nc.dram_tensor("sel", [2, 128], F32R, kind="ExternalInput").ap()
    y = nc.dram_tensor("y", [N, C], F32, kind="ExternalOutput").ap()

    with tile.TileContext(nc) as tc, ExitStack() as ctx:
        per = ctx.enter_context(tc.tile_pool(name="per", bufs=1))
        xT_s = per.tile([128, CT, N], BF16, tag="xT")
        qT_s = per.tile([128, 2, N], BF16, tag="qT")
        kT_s = per.tile([128, 2, N], BF16, tag="kT")
        vh_s = per.tile([128, JT, HPG, DH + 1], BF16, tag="vh")
        aoT_s = per.tile([128, 2, N], BF16, tag="aoT")
        wq0_t = per.tile([128, CT, 128], BF16, tag="wq0")
        wq1_t = per.tile([128, CT, 128], BF16, tag="wq1")
        wk0_t = per.tile([128, CT, 128], BF16, tag="wk0")
        wk1_t = per.tile([128, CT, 128], BF16, tag="wk1")
        wv_t = per.tile([128, CT, DG], BF16, tag="wv")
        wp_t = per.tile([128, DG // 128, C], BF16, tag="wp")
        qbT_s = per.tile([128, 2], F32, tag="qbT")
        ones_s = per.tile([1, 512], BF16, tag="ones")
        sel_s = per.tile([2, 128], F32R, tag="sel")
        warm = per.tile([1, 16], F32, tag="warm")

        # ---- DMA emission (order = per-queue priority; x is host-tiled
        # so every chunk is one fully-contiguous descriptor) ---------------
        QS = {"sync": nc.sync, "gp": nc.gpsimd, "sc": nc.scalar}

        def xdma(q, ct, nck):
            QS[q].dma_start(xT_s[:, ct, nck * 512:(nck + 1) * 512],
                            xT[ct, nck])

        nc.sync.dma_start(ones_s[:], ones)
        nc.sync.dma_start(wk0_t[:], wk0)
        nc.gpsimd.dma_start(wq0_t[:], wq0)
        nc.scalar.dma_start(qbT_s[:], qbT)
        nc.scalar.dma_start(sel_s[:], sel)
        # first key/query chunk: ct-ordered round-robin over the queues so
        # the production matmuls (which consume in ct order) start early
        for ct, q in enumerate(["sync", "sync", "sync", "gp", "gp", "gp",
                                "sc", "sc"]):
            xdma(q, ct, 0)
        for ct in range(4):
            xdma("sync", ct, 1)
        for ct in range(4, 8):
            xdma("gp", ct, 1)
        nc.scalar.dma_start(wv_t[:], wv)
        for nck in (2, 3):
            for ct in range(4):
                xdma("sync", ct, nck)
            for ct in range(4, 8):
                xdma("gp", ct, nck)
        nc.scalar.dma_start(wk1_t[:], wk1)
        nc.scalar.dma_start(wq1_t[:], wq1)
        nc.scalar.dma_start(wp_t[:], wp)

        with tc.tile_pool(name="es", bufs=20) as esp, \
             tc.tile_pool(name="sm", bufs=2) as sm2, \
             tc.tile_pool(name="yp", bufs=2) as yp, \
             tc.tile_pool(name="psA", bufs=2, space="PSUM") as psA, \
             tc.tile_pool(name="psS", bufs=2, space="PSUM") as psS, \
             tc.tile_pool(name="psB", bufs=2, space="PSUM") as psB:

            # warm the exp table while the bulk DMAs run, and spin the
            # PE clock up to full rate (dense dummy matmuls overlap the
            # first x-chunk DMAs, so production starts warm)
            nc.scalar.activation(warm[:], ones_s[:, 0:16], AF.Exp)
            jk0 = psA.tile([128, 512], F32, tag="mm", name="jk0")
            for _ in range(8):
                nc.tensor.matmul(jk0[:], ones_s[:, 0:128], ones_s[:],
                                 start=True, stop=True)
            nc.gpsimd.memset(vh_s[:, :, :, DH], 1.0)

            def qk_chunk(w_t, dst, dt, nck, bias=False):
                ps = psA.tile([128, 512], F32, tag="mm")
                for ct in range(CT):
                    nc.tensor.matmul(
                        ps[:], w_t[:, ct, :],
                        xT_s[:, ct, nck * 512:(nck + 1) * 512],
                        start=(ct == 0), stop=(ct == CT - 1))
                out = dst[:, dt, nck * 512:(nck + 1) * 512]
                if bias:
                    nc.vector.tensor_scalar_add(out, ps[:],
                                                qbT_s[:, dt:dt + 1])
                else:
                    nc.vector.tensor_copy(out, ps[:])

            def vhat(jt):
                ps = psA.tile([128, 512], F32, tag="mm")
                for ct in range(CT):
                    nc.tensor.matmul(ps[:, 0:DG],
                                     xT_s[:, ct, jt * 128:(jt + 1) * 128],
                                     wv_t[:, ct, :],
                                     start=(ct == 0), stop=(ct == CT - 1))
                for h in range(HPG):
                    nc.vector.tensor_copy(vh_s[:, jt, h, 0:DH],
                                          ps[:, h * DH:(h + 1) * DH])

            es_store = {}
            av_tiles = {}

            def se(p, ic, jcs):
                # S^T + exp for key tiles jcs of query chunk ic
                i0 = ic * 512
                for jc in jcs:
                    st = psS.tile([128, 1024], F32, tag="st")
                    nc.tensor.matmul(st[:, 0:512],
                                     kT_s[0:64, p, jc * 128:(jc + 1) * 128],
                                     qT_s[0:64, p, i0:i0 + 512],
                                     start=True, stop=True)
                    nc.tensor.matmul(st[:, 512:1024],
                                     kT_s[64:128, p, jc * 128:(jc + 1) * 128],
                                     qT_s[64:128, p, i0:i0 + 512],
                                     start=True, stop=True)
                    es = esp.tile([128, 1024], BF16, tag="es",
                                  name=f"es{p}_{ic}_{jc}")
                    es_store[(p, ic, jc)] = es
                    nc.scalar.activation(es[:], st[:], AF.Exp, scale=SCALE)

            def av(p, ic, blk):
                if (p, ic) not in av_tiles:
                    av_tiles[(p, ic)] = [
                        psB.tile([DH + 1, 512], F32, tag="outT",
                                 name=f"o{p}_{ic}{s}") for s in "ab"]
                outs = av_tiles[(p, ic)]
                for jc in range(4 * blk, 4 * blk + 4):
                    es = es_store.pop((p, ic, jc))
                    for h in range(2):
                        nc.tensor.matmul(
                            outs[h][:], vh_s[:, jc, 2 * p + h, :],
                            es[:, h * 512:(h + 1) * 512],
                            start=(jc == 0), stop=(jc == JT - 1))

            def norm(p, ic, act_assist=False):
                # PSUM-releasing copies first (frees psB for the next
                # chunk's AVs quickly), then recip/broadcast/multiply
                i0 = ic * 512
                outs = av_tiles.pop((p, ic))
                aos, dens = [], []
                for hi, outT in enumerate(outs):
                    ao = aoT_s[hi * 64:hi * 64 + 64, p, i0:i0 + 512]
                    den = sm2.tile([1, 512], F32, tag="den")
                    nc.vector.tensor_copy(ao, outT[0:64, :])
                    nc.vector.tensor_copy(den[:], outT[64:65, :])
                    aos.append(ao)
                    dens.append(den)
                recs = []
                for hi in range(2):
                    rec = sm2.tile([1, 512], F32, tag="rec")
                    nc.vector.reciprocal_approx_fast(rec[:], dens[hi][:])
                    rec_r = sm2.tile([1, 512], F32R, tag="rec_r")
                    nc.vector.tensor_copy(rec_r[:], rec[:])
                    recs.append(rec_r)
                bcs = []
                for hi in range(2):
                    bc = psA.tile([128, 512], F32, tag="mm")
                    nc.tensor.matmul(bc[0:64, :], sel_s[0:1, 0:64], recs[hi][:],
                                     start=True, stop=True)
                    bcs.append(bc)
                for hi in range(2):
                    nc.vector.tensor_mul(aos[hi], aos[hi], bcs[hi][0:64, :])

            def proj_it(it):
                # one 128-row tile of y: y[it] = aoT[:, :, it].T @ wp
                pss = [psA.tile([128, 512], F32, tag="mm", name=f"pj{it}_{e}")
                       for e in range(2)]
                for dt in range(DG // 128):
                    for ec in range(2):
                        nc.tensor.matmul(
                            pss[ec][:],
                            aoT_s[:, dt, it * 128:(it + 1) * 128],
                            wp_t[:, dt, ec * 512:(ec + 1) * 512],
                            start=(dt == 0), stop=(dt == DG // 128 - 1))
                yt = yp.tile([128, C], F32, tag="y")
                nc.vector.tensor_copy(yt[:, 0:512], pss[0][:])
                nc.vector.tensor_copy(yt[:, 512:1024], pss[1][:])
                eng = nc.sync if it % 2 == 0 else nc.gpsimd
                eng.dma_start(y[it * 128:(it + 1) * 128, :], yt[:])

            BLKS = [list(range(4 * b, 4 * b + 4)) for b in range(4)]

            # ---- emission schedule ---------------------------------------
            # buildup: S+exp streams run ahead (no vhat needed); AV trails.
            qk_chunk(wk0_t, kT_s, 0, 0)
            qk_chunk(wq0_t, qT_s, 0, 0, bias=True)
            se(0, 0, BLKS[0])
            qk_chunk(wk0_t, kT_s, 0, 1)
            se(0, 0, BLKS[1])
            qk_chunk(wq0_t, qT_s, 0, 1, bias=True)
            se(0, 1, BLKS[0])
            qk_chunk(wk0_t, kT_s, 0, 2)
            se(0, 0, BLKS[2])
            se(0, 1, BLKS[1])
            qk_chunk(wk0_t, kT_s, 0, 3)
            vhat(0), vhat(1), vhat(2), vhat(3)
            se(0, 0, BLKS[3])
            av(0, 0, 0)
            vhat(4), vhat(5), vhat(6), vhat(7)
            qk_chunk(wq0_t, qT_s, 0, 2, bias=True)
            se(0, 1, BLKS[2])
            av(0, 0, 1)
            vhat(8), vhat(9), vhat(10), vhat(11)
            qk_chunk(wq0_t, qT_s, 0, 3, bias=True)
            se(0, 1, BLKS[3])
            av(0, 0, 2)
            vhat(12), vhat(13), vhat(14), vhat(15)
            se(0, 2, BLKS[0])
            av(0, 0, 3)
            norm(0, 0)

            # steady state: se leads, av trails ~2 blocks (catching up from
            # the buildup deficit), pair-1 q/k production and the projection
            # fill the remaining PE slack.  Production must be emitted
            # before its consuming se (same in-order PE queue).
            fill = [
                lambda: qk_chunk(wk1_t, kT_s, 1, 0),
                lambda: qk_chunk(wk1_t, kT_s, 1, 1),
                lambda: qk_chunk(wq1_t, qT_s, 1, 0, bias=True),
                lambda: qk_chunk(wk1_t, kT_s, 1, 2),
                lambda: qk_chunk(wk1_t, kT_s, 1, 3),
                lambda: qk_chunk(wq1_t, qT_s, 1, 1, bias=True),
                lambda: qk_chunk(wq1_t, qT_s, 1, 2, bias=True),
                lambda: qk_chunk(wq1_t, qT_s, 1, 3, bias=True),
            ]
            seq = [(0, 2, 1), (0, 2, 2), (0, 2, 3),
                   (0, 3, 0), (0, 3, 1), (0, 3, 2), (0, 3, 3),
                   (1, 0, 0), (1, 0, 1), (1, 0, 2), (1, 0, 3),
                   (1, 1, 0), (1, 1, 1), (1, 1, 2), (1, 1, 3),
                   (1, 2, 0), (1, 2, 1), (1, 2, 2), (1, 2, 3),
                   (1, 3, 0), (1, 3, 1), (1, 3, 2), (1, 3, 3)]
            avq = [(p, ic, b)
                   for (p, ic) in [(0, 1), (0, 2), (0, 3),
                                   (1, 0), (1, 1), (1, 2), (1, 3)]
                   for b in range(4)]
            proj_pending = []
            se_done = {(0, 0, b) for b in range(4)}
            se_done |= {(0, 1, b) for b in range(4)}
            se_done.add((0, 2, 0))

            def pop_av():
                ap_, ai_, ab_ = avq.pop(0)
                av(ap_, ai_, ab_)
                if ab_ == 3:
                    norm(ap_, ai_)
                    if ap_ == 1:
                        proj_pending.extend(range(4 * ai_, 4 * ai_ + 4))

            last_norm = avq[-1][:2]
            fi = 0
            for si, (pi, ici, blk) in enumerate(seq):
                if fi < len(fill) and si % 2 == 1:
                    fill[fi]()        # before se: consumers must trail
                    fi += 1
                se(pi, ici, BLKS[blk])
                se_done.add((pi, ici, blk))
                pop_av()
                # catch up: early (buildup deficit) and twice late (shrink
                # the drain tail), bounded by what se has emitted
                if (si < 3 or si in (16, 19)) and avq \
                        and tuple(avq[0]) in se_done:
                    pop_av()
                if proj_pending:
                    proj_it(proj_pending.pop(0))
            while avq:
                pop_av()
            while proj_pending:
                proj_it(proj_pending.pop(0))

    nc.compile()
    return nc


def _get_nc():
    if "nc" not in _CACHE:
        _CACHE["nc"] = _build()
    return _CACHE["nc"]


def kernel(x, qkv_w, qkv_b, proj_w, proj_b):
    global LAST_RESULTS
    x = np.asarray(x, dtype=np.float32)
    qkv_w = np.asarray(qkv_w, dtype=np.float32)
    qkv_b = np.asarray(qkv_b, dtype=np.float32)
    proj_w = np.asarray(proj_w, dtype=np.float32)
    proj_b = np.asarray(proj_b, dtype=np.float32)

    nc = _get_nc()
    bf16 = ml_dtypes.bfloat16

    wqT_f = qkv_w[0:C].T                # [C, C]
    wkT_f = qkv_w[C:2 * C].T
    wvT_f = qkv_w[2 * C:3 * C].T
    wpT_f = proj_w.T                    # [C, C]

    def tile128(a):
        # [C, W] -> [128, CT, W] with partition = c % 128, ct = c // 128
        w = a.shape[1]
        return np.ascontiguousarray(
            a.reshape(CT, 128, w).transpose(1, 0, 2))

    in_maps = []
    for c in range(NCORES):
        b, g = divmod(c, HG)
        ds = g * DG
        wq_g = tile128(wqT_f[:, ds:ds + DG]).astype(bf16)  # [128, CT, 256]
        wk_g = tile128(wkT_f[:, ds:ds + DG]).astype(bf16)
        wp_g = np.ascontiguousarray(
            wpT_f[ds:ds + DG].reshape(2, 128, C).transpose(1, 0, 2)).astype(bf16)
        # qbT: per-partition q bias, column dt = head pair
        qbT = np.ascontiguousarray(
            qkv_b[ds:ds + DG].reshape(2, 128).T, dtype=np.float32)
        sel_a = np.zeros((2, 128), np.float32)
        sel_a[0, 0:64] = 1.0
        sel_a[1, 64:128] = 1.0
        in_maps.append({
            "xT": np.ascontiguousarray(
                x[b].T.reshape(CT, 128, IC, 512).transpose(0, 2, 1, 3)
            ).astype(bf16),
            "wq0": np.ascontiguousarray(wq_g[:, :, 0:128]),
            "wq1": np.ascontiguousarray(wq_g[:, :, 128:256]),
            "wk0": np.ascontiguousarray(wk_g[:, :, 0:128]),
            "wk1": np.ascontiguousarray(wk_g[:, :, 128:256]),
            "wv": tile128(wvT_f[:, ds:ds + DG]).astype(bf16),
            "wp": wp_g,
            "qbT": qbT,
            "ones": np.ones((1, 512), bf16),
            "sel": sel_a,
        })

    LAST_RESULTS = run_bass_kernel_spmd(nc, in_maps, list(range(NCORES)))
    # host unshard: sum the 4 partial projections per batch and add the
    # folded bias (proj_b + v_bias @ proj_w.T -- exact, since sum(attn)=1)
    out_bias = proj_b + qkv_b[2 * C:3 * C] @ proj_w.T
    out = np.empty((B, N, C), np.float32)
    for b in range(B):
        acc = LAST_RESULTS.results[b * HG]["y"].astype(np.float32)
        for g in range(1, HG):
            acc = acc + LAST_RESULTS.results[b * HG + g]["y"]
        out[b] = acc + out_bias
    return out


# revision 32
# speedup vs baseline: 1.1648x; 1.0077x over previous
"""Multi-head attention forward on 8 Trainium2 NeuronCores.

Reference computation (B=2, N=2048, C=1024, H=16, Dh=64):
    qkv = x @ qkv_w.T + qkv_b            -> q, k, v per head
    attn = softmax(q @ k.T / sqrt(Dh))
    out  = (attn @ v) reassembled, then out @ proj_w.T + proj_b

Sharding: 8 cores = 2 batches x 4 head groups (tensor parallel on heads,
data parallel on batch).  Each core computes q/k/v for its 4 heads over
its batch's 2048 tokens, attention for those heads, and a partial
projection with its head-group's rows of proj_w.  The host sums the 4
partial projections per batch and adds the (host-folded) proj + v biases.

Schedule: everything runs in the S^T orientation (S^T[j,i] = sum_d
kT[d,j] qT[d,i]) so softmax reductions over keys happen via matmul -- a
ones column in v-hat yields the denominator as row 64 of the AV PSUM
tile.  The kernel is ACT(exp)-bound, so the S+exp stream is decoupled
from the AV stream: exp'd score tiles (es) buffer in SBUF, letting exp
run ahead across query chunks while q/k/v production and the projection
back-fill PE slack.  All matmul operands are bf16.
Softmax max-subtraction is skipped (S ~ N(0,1)).  The k bias is
dropped (softmax-invariant); v/proj biases are folded on the host.
"""

import sys

if "/opt/trn_rl_repo" not in sys.path:
    sys.path.insert(0, "/opt/trn_rl_repo")

from contextlib import ExitStack

import ml_dtypes
import numpy as np

from concourse import bacc, mybir, tile
from concourse.bass_utils import run_bass_kernel_spmd

F32 = mybir.dt.float32
F32R = mybir.dt.float32r
BF16 = mybir.dt.bfloat16
AF = mybir.ActivationFunctionType

B, N, C, H, DH = 2, 2048, 1024, 16, 64
NCORES = 8
HG = 4              # head groups (cores per batch)
HPG = H // HG       # 4 heads per core
DG = HPG * DH       # 256 projected dims per core
CT = C // 128       # 8 contraction tiles
JT = N // 128       # 16 key tiles
IC = N // 512       # 4 query chunks
SCALE = DH ** -0.5

_CACHE = {}
LAST_RESULTS = None


def _build():
    nc = bacc.Bacc("TRN2", target_bir_lowering=False, debug=False,
                   num_devices=NCORES)

    xT = nc.dram_tensor("xT", [CT, 2, 128, 1024], BF16, kind="ExternalInput").ap()
    wq0 = nc.dram_tensor("wq0", [128, CT, 128], BF16, kind="ExternalInput").ap()
    wq1 = nc.dram_tensor("wq1", [128, CT, 128], BF16, kind="ExternalInput").ap()
    wk0 = nc.dram_tensor("wk0", [128, CT, 128], BF16, kind="ExternalInput").ap()
    wk1 = nc.dram_tensor("wk1", [128, CT, 128], BF16, kind="ExternalInput").ap()
    wv = nc.dram_tensor("wv", [128, CT, DG], BF16, kind="ExternalInput").ap()
    wp = nc.dram_tensor("wp", [128, DG // 128, C], BF16, kind="ExternalInput").ap()
    qbT = nc.dram_tensor("qbT", [128, 2], F32, kind="ExternalInput").ap()
    ones = nc.dram_tensor("ones", [1, 512], BF16, kind="ExternalInput").ap()
    sel = nc.dram_tensor("sel", [2, 128], F32R, kind="ExternalInput").ap()
    y = nc.dram_tensor("y", [N, C], F32, kind="ExternalOutput").ap()

    with tile.TileContext(nc) as tc, ExitStack() as ctx:
        per = ctx.enter_context(tc.tile_pool(name="per", bufs=1))
        xT_s = per.tile([128, CT, N], BF16, tag="xT")
        qT_s = per.tile([128, 2, N], BF16, tag="qT")
        kT_s = per.tile([128, 2, N], BF16, tag="kT")
        vh_s = per.tile([128, JT, HPG, DH + 1], BF16, tag="vh")
        aoT_s = per.tile([128, 2, N], BF16, tag="aoT")
        wq0_t = per.tile([128, CT, 128], BF16, tag="wq0")
        wq1_t = per.tile([128, CT, 128], BF16, tag="wq1")
        wk0_t = per.tile([128, CT, 128], BF16, tag="wk0")
        wk1_t = per.tile([128, CT, 128], BF16, tag="wk1")
        wv_t = per.tile([128, CT, DG], BF16, tag="wv")
        wp_t = per.tile([128, DG // 128, C], BF16, tag="wp")
        qbT_s = per.tile([128, 2], F32, tag="qbT")
        ones_s = per.tile([1, 512], BF16, tag="ones")
        sel_s = per.tile([2, 128], F32R, tag="sel")
        warm = per.tile([1, 16], F32, tag="warm")

        # ---- DMA emission (order = per-queue priority; x is host-tiled
        # so every chunk is one fully-contiguous descriptor) ---------------
        QS = {"sync": nc.sync, "gp": nc.gpsimd, "sc": nc.scalar}

        def xdma(q, ct, half):
            # token-pair halves: 2KB per-partition lines (DMA-efficient)
            QS[q].dma_start(xT_s[:, ct, half * 1024:(half + 1) * 1024],
                            xT[ct, half])

        nc.sync.dma_start(ones_s[:], ones)
        nc.sync.dma_start(wk0_t[:], wk0)
        nc.gpsimd.dma_start(wq0_t[:], wq0)
        nc.scalar.dma_start(qbT_s[:], qbT)
        nc.scalar.dma_start(sel_s[:], sel)
        # first half of x (keys/queries 0:1024) spread over all queues
        for ct, q in enumerate(["sync", "sync", "sync", "gp", "gp", "gp",
                                "sc", "sc"]):
            xdma(q, ct, 0)
        nc.scalar.dma_start(wv_t[:], wv)
        for ct in range(4):
            xdma("sync", ct, 1)
        for ct in range(4, 8):
            xdma("gp", ct, 1)
        nc.scalar.dma_start(wk1_t[:], wk1)
        nc.scalar.dma_start(wq1_t[:], wq1)
        nc.scalar.dma_start(wp_t[:], wp)

        with tc.tile_pool(name="es", bufs=20) as esp, \
             tc.tile_pool(name="sm", bufs=2) as sm2, \
             tc.tile_pool(name="yp", bufs=2) as yp, \
             tc.tile_pool(name="psA", bufs=2, space="PSUM") as psA, \
             tc.tile_pool(name="psS", bufs=2, space="PSUM") as psS, \
             tc.tile_pool(name="psB", bufs=2, space="PSUM") as psB:

            # warm the exp table while the bulk DMAs run, and spin the
            # PE clock up to full rate (dense dummy matmuls overlap the
            # first x-chunk DMAs, so production starts warm)
            nc.scalar.activation(warm[:], ones_s[:, 0:16], AF.Exp)
            jk0 = psA.tile([128, 512], F32, tag="mm", name="jk0")
            for _ in range(8):
                nc.tensor.matmul(jk0[:], ones_s[:, 0:128], ones_s[:],
                                 start=True, stop=True)
            nc.gpsimd.memset(vh_s[:, :, :, DH], 1.0)

            def qk_chunk(w_t, dst, dt, nck, bias=False):
                ps = psA.tile([128, 512], F32, tag="mm")
                for ct in range(CT):
                    nc.tensor.matmul(
                        ps[:], w_t[:, ct, :],
                        xT_s[:, ct, nck * 512:(nck + 1) * 512],
                        start=(ct == 0), stop=(ct == CT - 1))
                out = dst[:, dt, nck * 512:(nck + 1) * 512]
                if bias:
                    nc.vector.tensor_scalar_add(out, ps[:],
                                                qbT_s[:, dt:dt + 1])
                else:
                    nc.vector.tensor_copy(out, ps[:])

            def vhat(jt):
                ps = psA.tile([128, 512], F32, tag="mm")
                for ct in range(CT):
                    nc.tensor.matmul(ps[:, 0:DG],
                                     xT_s[:, ct, jt * 128:(jt + 1) * 128],
                                     wv_t[:, ct, :],
                                     start=(ct == 0), stop=(ct == CT - 1))
                for h in range(HPG):
                    nc.vector.tensor_copy(vh_s[:, jt, h, 0:DH],
                                          ps[:, h * DH:(h + 1) * DH])

            es_store = {}
            av_tiles = {}

            def se(p, ic, jcs):
                # S^T + exp for key tiles jcs of query chunk ic
                i0 = ic * 512
                for jc in jcs:
                    st = psS.tile([128, 1024], F32, tag="st")
                    nc.tensor.matmul(st[:, 0:512],
                                     kT_s[0:64, p, jc * 128:(jc + 1) * 128],
                                     qT_s[0:64, p, i0:i0 + 512],
                                     start=True, stop=True)
                    nc.tensor.matmul(st[:, 512:1024],
                                     kT_s[64:128, p, jc * 128:(jc + 1) * 128],
                                     qT_s[64:128, p, i0:i0 + 512],
                                     start=True, stop=True)
                    es = esp.tile([128, 1024], BF16, tag="es",
                                  name=f"es{p}_{ic}_{jc}")
                    es_store[(p, ic, jc)] = es
                    nc.scalar.activation(es[:], st[:], AF.Exp, scale=SCALE)

            def av(p, ic, blk):
                if (p, ic) not in av_tiles:
                    av_tiles[(p, ic)] = [
                        psB.tile([DH + 1, 512], F32, tag="outT",
                                 name=f"o{p}_{ic}{s}") for s in "ab"]
                outs = av_tiles[(p, ic)]
                for jc in range(4 * blk, 4 * blk + 4):
                    es = es_store.pop((p, ic, jc))
                    for h in range(2):
                        nc.tensor.matmul(
                            outs[h][:], vh_s[:, jc, 2 * p + h, :],
                            es[:, h * 512:(h + 1) * 512],
                            start=(jc == 0), stop=(jc == JT - 1))

            def norm(p, ic, act_assist=False):
                # PSUM-releasing copies first (frees psB for the next
                # chunk's AVs quickly), then recip/broadcast/multiply
                i0 = ic * 512
                outs = av_tiles.pop((p, ic))
                aos, dens = [], []
                for hi, outT in enumerate(outs):
                    ao = aoT_s[hi * 64:hi * 64 + 64, p, i0:i0 + 512]
                    den = sm2.tile([1, 512], F32, tag="den")
                    nc.vector.tensor_copy(ao, outT[0:64, :])
                    nc.vector.tensor_copy(den[:], outT[64:65, :])
                    aos.append(ao)
                    dens.append(den)
                recs = []
                for hi in range(2):
                    rec = sm2.tile([1, 512], F32, tag="rec")
                    nc.vector.reciprocal_approx_fast(rec[:], dens[hi][:])
                    rec_r = sm2.tile([1, 512], F32R, tag="rec_r")
                    nc.vector.tensor_copy(rec_r[:], rec[:])
                    recs.append(rec_r)
                bcs = []
                for hi in range(2):
                    bc = psA.tile([128, 512], F32, tag="mm")
                    nc.tensor.matmul(bc[0:64, :], sel_s[0:1, 0:64], recs[hi][:],
                                     start=True, stop=True)
                    bcs.append(bc)
                for hi in range(2):
                    nc.vector.tensor_mul(aos[hi], aos[hi], bcs[hi][0:64, :])

            def proj_it(it):
                # one 128-row tile of y: y[it] = aoT[:, :, it].T @ wp
                pss = [psA.tile([128, 512], F32, tag="mm", name=f"pj{it}_{e}")
                       for e in range(2)]
                for dt in range(DG // 128):
                    for ec in range(2):
                        nc.tensor.matmul(
                            pss[ec][:],
                            aoT_s[:, dt, it * 128:(it + 1) * 128],
                            wp_t[:, dt, ec * 512:(ec + 1) * 512],
                            start=(dt == 0), stop=(dt == DG // 128 - 1))
                yt = yp.tile([128, C], F32, tag="y")
                nc.vector.tensor_copy(yt[:, 0:512], pss[0][:])
                nc.vector.tensor_copy(yt[:, 512:1024], pss[1][:])
                eng = nc.sync if it % 2 == 0 else nc.gpsimd
                eng.dma_start(y[it * 128:(it + 1) * 128, :], yt[:])

            BLKS = [list(range(4 * b, 4 * b + 4)) for b in range(4)]

            # ---- emission schedule ---------------------------------------
            # buildup: S+exp streams run ahead (no vhat needed); AV trails.
            qk_chunk(wk0_t, kT_s, 0, 0)
            qk_chunk(wq0_t, qT_s, 0, 0, bias=True)
            se(0, 0, BLKS[0])
            qk_chunk(wk0_t, kT_s, 0, 1)
            se(0, 0, BLKS[1])
            qk_chunk(wq0_t, qT_s, 0, 1, bias=True)
            se(0, 1, BLKS[0])
            qk_chunk(wk0_t, kT_s, 0, 2)
            se(0, 0, BLKS[2])
            se(0, 1, BLKS[1])
            qk_chunk(wk0_t, kT_s, 0, 3)
            vhat(0), vhat(1), vhat(2), vhat(3)
            se(0, 0, BLKS[3])
            av(0, 0, 0)
            vhat(4), vhat(5), vhat(6), vhat(7)
            qk_chunk(wq0_t, qT_s, 0, 2, bias=True)
            se(0, 1, BLKS[2])
            av(0, 0, 1)
            vhat(8), vhat(9), vhat(10), vhat(11)
            qk_chunk(wq0_t, qT_s, 0, 3, bias=True)
            se(0, 1, BLKS[3])
            av(0, 0, 2)
            vhat(12), vhat(13), vhat(14), vhat(15)
            se(0, 2, BLKS[0])
            av(0, 0, 3)
            norm(0, 0)

            # steady state: se leads, av trails ~2 blocks (catching up from
            # the buildup deficit), pair-1 q/k production and the projection
            # fill the remaining PE slack.  Production must be emitted
            # before its consuming se (same in-order PE queue).
            fill = [
                lambda: qk_chunk(wk1_t, kT_s, 1, 0),
                lambda: qk_chunk(wk1_t, kT_s, 1, 1),
                lambda: qk_chunk(wq1_t, qT_s, 1, 0, bias=True),
                lambda: qk_chunk(wk1_t, kT_s, 1, 2),
                lambda: qk_chunk(wk1_t, kT_s, 1, 3),
                lambda: qk_chunk(wq1_t, qT_s, 1, 1, bias=True),
                lambda: qk_chunk(wq1_t, qT_s, 1, 2, bias=True),
                lambda: qk_chunk(wq1_t, qT_s, 1, 3, bias=True),
            ]
            seq = [(0, 2, 1), (0, 2, 2), (0, 2, 3),
                   (0, 3, 0), (0, 3, 1), (0, 3, 2), (0, 3, 3),
                   (1, 0, 0), (1, 0, 1), (1, 0, 2), (1, 0, 3),
                   (1, 1, 0), (1, 1, 1), (1, 1, 2), (1, 1, 3),
                   (1, 2, 0), (1, 2, 1), (1, 2, 2), (1, 2, 3),
                   (1, 3, 0), (1, 3, 1), (1, 3, 2), (1, 3, 3)]
            avq = [(p, ic, b)
                   for (p, ic) in [(0, 1), (0, 2), (0, 3),
                                   (1, 0), (1, 1), (1, 2), (1, 3)]
                   for b in range(4)]
            proj_pending = []
            se_done = {(0, 0, b) for b in range(4)}
            se_done |= {(0, 1, b) for b in range(4)}
            se_done.add((0, 2, 0))

            def pop_av():
                ap_, ai_, ab_ = avq.pop(0)
                av(ap_, ai_, ab_)
                if ab_ == 3:
                    norm(ap_, ai_)
                    if ap_ == 1:
                        proj_pending.extend(range(4 * ai_, 4 * ai_ + 4))

            last_norm = avq[-1][:2]
            fi = 0
            for si, (pi, ici, blk) in enumerate(seq):
                if fi < len(fill) and si % 2 == 1:
                    fill[fi]()        # before se: consumers must trail
                    fi += 1
                se(pi, ici, BLKS[blk])
                se_done.add((pi, ici, blk))
                pop_av()
                # catch up: early (buildup deficit) and twice late (shrink
                # the drain tail), bounded by what se has emitted
                if (si < 3 or si in (16, 19)) and avq \
                        and tuple(avq[0]) in se_done:
                    pop_av()
                if proj_pending:
                    proj_it(proj_pending.pop(0))
            while avq:
                pop_av()
            while proj_pending:
                proj_it(proj_pending.pop(0))

    nc.compile()
    return nc


def _get_nc():
    if "nc" not in _CACHE:
        _CACHE["nc"] = _build()
    return _CACHE["nc"]


def kernel(x, qkv_w, qkv_b, proj_w, proj_b):
    global LAST_RESULTS
    x = np.asarray(x, dtype=np.float32)
    qkv_w = np.asarray(qkv_w, dtype=np.float32)
    qkv_b = np.asarray(qkv_b, dtype=np.float32)
    proj_w = np.asarray(proj_w, dtype=np.float32)
    proj_b = np.asarray(proj_b, dtype=np.float32)

    nc = _get_nc()
    bf16 = ml_dtypes.bfloat16

    wqT_f = qkv_w[0:C].T                # [C, C]
    wkT_f = qkv_w[C:2 * C].T
    wvT_f = qkv_w[2 * C:3 * C].T
    wpT_f = proj_w.T                    # [C, C]

    def tile128(a):
        # [C, W] -> [128, CT, W] with partition = c % 128, ct = c // 128
        w = a.shape[1]
        return np.ascontiguousarray(
            a.reshape(CT, 128, w).transpose(1, 0, 2))

    in_maps = []
    for c in range(NCORES):
        b, g = divmod(c, HG)
        ds = g * DG
        wq_g = tile128(wqT_f[:, ds:ds + DG]).astype(bf16)  # [128, CT, 256]
        wk_g = tile128(wkT_f[:, ds:ds + DG]).astype(bf16)
        wp_g = np.ascontiguousarray(
            wpT_f[ds:ds + DG].reshape(2, 128, C).transpose(1, 0, 2)).astype(bf16)
        # qbT: per-partition q bias, column dt = head pair
        qbT = np.ascontiguousarray(
            qkv_b[ds:ds + DG].reshape(2, 128).T, dtype=np.float32)
        sel_a = np.zeros((2, 128), np.float32)
        sel_a[0, 0:64] = 1.0
        sel_a[1, 64:128] = 1.0
        in_maps.append({
            "xT": np.ascontiguousarray(
                x[b].T.reshape(CT, 128, 2, 1024).transpose(0, 2, 1, 3)
            ).astype(bf16),
            "wq0": np.ascontiguousarray(wq_g[:, :, 0:128]),
            "wq1": np.ascontiguousarray(wq_g[:, :, 128:256]),
            "wk0": np.ascontiguousarray(wk_g[:, :, 0:128]),
            "wk1": np.ascontiguousarray(wk_g[:, :, 128:256]),
            "wv": tile128(wvT_f[:, ds:ds + DG]).astype(bf16),
            "wp": wp_g,
            "qbT": qbT,
            "ones": np.ones((1, 512), bf16),
            "sel": sel_a,
        })

    LAST_RESULTS = run_bass_kernel_spmd(nc, in_maps, list(range(NCORES)))
    # host unshard: sum the 4 partial projections per batch and add the
    # folded bias (proj_b + v_bias @ proj_w.T -- exact, since sum(attn)=1)
    out_bias = proj_b + qkv_b[2 * C:3 * C] @ proj_w.T
    out = np.empty((B, N, C), np.float32)
    for b in range(B):
        acc = LAST_RESULTS.results[b * HG]["y"].astype(np.float32)
        for g in range(1, HG):
            acc = acc + LAST_RESULTS.results[b * HG + g]["y"]
        out[b] = acc + out_bias
    return out
